# revision 38
# baseline (speedup 1.0000x reference)
"""Multi-head attention (b=16, l=1025, d=768, H=12) on 8 TRN2 NeuronCores.

Sharding: data-parallel over batch - 2 batch elements per core, no
collectives.

Per-core kernel. Key ideas vs the v1 baseline (357us -> 260us):

1. fp8 DoubleRow matmuls (0.5 cycles/row) for scores and the QKV
   projections:
   - Projections use an error-compensated 3-term split: X = Xhi+Xlo,
     W = Whi+Wlo (both fp8, W pre-scaled by 64 against e4m3 subnormals),
     computed as XhiWhi + XloWhi + XhiWlo over k-tile-pair planes -
     0.75x the bf16 cost at bf16-level accuracy. The 64x output scale is
     absorbed downstream (1/64 in the Q/K psum drain; V stays 64-scaled
     through PV and cancels via the softmax denominator, whose V_aug
     ones-column is 64.0).
   - Scores are one-sided fp8: S^T = 2*(Khi^T.Qhi) via stride-0 planes,
     with the doubling folded into the exp scale. The K bias is dropped
     (softmax over keys is invariant per query); the Q bias is folded
     into Qhi.
2. PV in O-orientation: out O[q, 65] per (head, q-block, key-block),
   lhsT = P^T slice, rhs = V_aug [128 keys, 64+1] whose last column
   makes output col 64 the softmax denominator. Full contraction, full
   output partitions, free dim 65: half the cost of the O^T orientation
   with no separate denominator pass. PSUM accumulation groups sharing
   a bank are kept consecutive (start=True clears the whole bank's
   has_written bits on hardware).
3. Cross-head software pipeline: head h's DoubleRow-score/exp loop
   carries head h-1's PV groups (one q-block group per j step, two
   steps delayed so the accumulator-slot WAR on the DVE normalize never
   blocks the in-order PE queue). ACT (the exp engine, ~200us busy) is
   the pace-setter and runs gap-free through both elements' attention.
4. O^T for the output projection comes from the DMA xbar transpose
   (zero PE/ACT/DVE cost). Element 1 is processed in head order
   [10, 11, 0..9] with per-pair transposes into per-pair O^T tiles so
   only the final pair gates the tail output projection (k-order puts
   that matmul last in each accumulation group).
5. The l=1025 stragglers: query 1024 is handled entirely on the host
   from exported K/V; key 1024's rank-12 contribution to Y is added on
   the host from exported p8 = exp(s8/8) (computed on device as N=1
   matmuls + a Schraudolph bit-trick exp on DVE) and denominators.
6. Projection matmuls of the next element and the output projection of
   the previous one ride the attention loop as PE filler via
   generators; batched 3D-AP input DMAs plus thin early slices of Wqk
   (columns m=0,6) and a PE warm-up keep the head short.
"""

import contextlib

import numpy as np
import ml_dtypes

import concourse.bass as bass
import concourse.bacc as bacc
import concourse.mybir as mybir
import concourse.tile as tile
from concourse.bass_utils import run_bass_kernel_spmd

N_CORES = 8
B = 16
L = 1025
D = 768
H = 12
DH = 64
BPC = B // N_CORES
KT = D // 128   # 6 contraction tiles
NJ = 8          # full 128-key blocks; key 1024 handled via s8/p8
SCALE = 1.0 / np.sqrt(DH)
KLO = 1040      # Klo plane offset inside khilo tiles (16-aligned)
VW = DH + 1     # 65: V_aug block per head (ones column last)

BF16 = mybir.dt.bfloat16
F32 = mybir.dt.float32
FP8 = mybir.dt.float8e4
NPF8 = ml_dtypes.float8_e4m3
EXP = mybir.ActivationFunctionType.Exp
MULT = mybir.AluOpType.mult
ADD = mybir.AluOpType.add
SUB = mybir.AluOpType.subtract
DR = mybir.MatmulPerfMode.DoubleRow

_CACHE = {}


def _ap(t, poff, pcount, foff, fdims):
    """AP on tile t at partition offset poff (count pcount), free offset
    foff with free dims [(step, count), ...]."""
    base = t[:]
    pstep = base.ap[0][0]
    return bass.AP(tensor=base.tensor,
                   offset=base.offset + poff * pstep + foff,
                   ap=[[pstep, pcount]] + [list(d) for d in fdims])


def _dram3(dram_ap, psize, nt, tstride, inner):
    """3D dram AP: [[row, psize], [tstride, nt], [1, inner]]."""
    return bass.AP(tensor=dram_ap.tensor, offset=dram_ap.offset,
                   ap=[[inner, psize], [tstride, nt], [1, inner]])


def _build():
    nc = bacc.Bacc("TRN2", target_bir_lowering=False, debug=False,
                   num_devices=N_CORES)
    xT = nc.dram_tensor("xT", [BPC, 2, D, L], FP8, kind="ExternalInput")
    w_qk = nc.dram_tensor("w_qk", [2, D, 2 * D], FP8, kind="ExternalInput")
    w_v = nc.dram_tensor("w_v", [2, D, D], FP8, kind="ExternalInput")
    w_o = nc.dram_tensor("w_o", [D, D], BF16, kind="ExternalInput")
    b_q = nc.dram_tensor("b_q", [D, 1], F32, kind="ExternalInput")
    b_v = nc.dram_tensor("b_v", [1, D], F32, kind="ExternalInput")
    b_o = nc.dram_tensor("b_o", [D, 1], F32, kind="ExternalInput")
    yT = nc.dram_tensor("yT", [BPC, KT, 128, 1024], BF16,
                        kind="ExternalOutput")
    kTo = nc.dram_tensor("kTo", [BPC, KT, 128, 1040], FP8,
                         kind="ExternalOutput")
    vo = nc.dram_tensor("vo", [BPC, 9, 128, H * VW], BF16,
                        kind="ExternalOutput")
    p8o = nc.dram_tensor("p8o", [BPC, 128, 96], BF16, kind="ExternalOutput")
    dno = nc.dram_tensor("dno", [BPC, 128, 96], F32, kind="ExternalOutput")

    with tile.TileContext(nc) as tc:
        _emit(nc, tc, xT, w_qk, w_v, w_o, b_q, b_v, b_o, yT, kTo, vo, p8o,
              dno)
    nc.compile()
    return nc


def _emit(nc, tc, xT, w_qk, w_v, w_o, b_q, b_v, b_o, yT, kTo, vo, p8o, dno):
    ctx = contextlib.ExitStack()
    with ctx:
        consts = ctx.enter_context(tc.tile_pool(name="consts", bufs=1))
        xpool = ctx.enter_context(tc.tile_pool(name="xpool", bufs=2))
        qpool = ctx.enter_context(tc.tile_pool(name="qpool", bufs=2))
        kpool = ctx.enter_context(tc.tile_pool(name="kpool", bufs=2))
        vpool = ctx.enter_context(tc.tile_pool(name="vpool", bufs=2))
        ptpool = ctx.enter_context(tc.tile_pool(name="ptpool", bufs=16))
        osbpool = ctx.enter_context(tc.tile_pool(name="osbpool", bufs=2))
        otpool = ctx.enter_context(tc.tile_pool(name="otpool", bufs=1))
        recpool = ctx.enter_context(tc.tile_pool(name="recpool", bufs=3))
        p8pool = ctx.enter_context(tc.tile_pool(name="p8pool", bufs=2))
        ytpool = ctx.enter_context(tc.tile_pool(name="ytpool", bufs=4))
        # PSUM: scores 2x[128,1024]=4 banks, pv 2x[128,260]=2, proj 2x=2
        bigp = ctx.enter_context(tc.tile_pool(name="bigp", bufs=2,
                                              space="PSUM"))
        pvp = ctx.enter_context(tc.tile_pool(name="pvp", bufs=2,
                                             space="PSUM"))
        projp = ctx.enter_context(tc.tile_pool(name="projp", bufs=2,
                                               space="PSUM"))

        wqk_t = [consts.tile([128, KT * 2 * D], FP8, name=f"wqk{u}")
                 for u in (0, 1)]
        wv_t = [consts.tile([128, KT * D], FP8, name=f"wv{u}")
                for u in (0, 1)]
        wo_t = consts.tile([128, KT * D], BF16, name="wo")
        bq_t = consts.tile([128, KT], F32, name="bq")
        bo_t = consts.tile([128, KT], F32, name="bo")
        bv_bc = consts.tile([128, D], F32, name="bvbc")

        def wqkp(u, kp, a, n):
            """lhsT planes (k-pair kp, kp+1) of hi(0)/lo(1) wqk."""
            return _ap(wqk_t[u], 0, 128, kp * 2 * D + a,
                       [[2 * D, 2], [1, n]])

        def wvp(u, kp, a, n):
            return _ap(wv_t[u], 0, 128, kp * D + a, [[D, 2], [1, n]])

        def wo(k, a, n):
            return _ap(wo_t, 0, 128, k * D + a, [[1, n]])

        xt = {}
        qhi = {}
        khilo = {}
        vt = {}
        osb = {}
        oTt = {}
        p8 = {}
        dnF = {}

        def alloc_elem(e):
            qhi[e] = [qpool.tile([128, 1024], FP8, tag=f"qhi{m}",
                                 name=f"qhi{e}_{m}") for m in range(KT)]
            khilo[e] = [kpool.tile([128, 1040], FP8, tag=f"kh{m}",
                                   name=f"kh{e}_{m}") for m in range(KT)]
            vt[e] = [vpool.tile([128, H * VW], BF16, tag=f"vt{j}",
                                name=f"vt{e}_{j}") for j in range(9)]
            osb[e] = osbpool.tile([128, 8 * D], BF16, tag="osb",
                                  name=f"osb{e}")
            if e == 0:
                oTt[0] = otpool.tile([128, KT * 1024], BF16, tag="oT0",
                                     name="oT0")
            else:
                oTt[1] = [otpool.tile([128, 1024], BF16, tag=f"oT1_{k}",
                                      name=f"oT1_{k}") for k in range(KT)]
            p8[e] = p8pool.tile([128, 96], BF16, tag="p8", name=f"p8_{e}")
            dnF[e] = p8pool.tile([128, 96], F32, tag="dn", name=f"dn{e}")

        XS = 1040  # padded k-tile stride (DoubleRow plane step % 16 == 0)

        def xtp(e, u, kp, a, n):
            """planes (k-pair kp, kp+1) of hi(0)/lo(1) X^T."""
            return _ap(xt[e][u], 0, 128, kp * XS + a, [[XS, 2], [1, n]])

        def load_x(e):
            xt[e] = [xpool.tile([128, KT * XS], FP8, tag=f"xt{u}",
                                name=f"xt{e}_{u}") for u in (0, 1)]
            for u in (0, 1):
                nc.sync.dma_start(
                    out=_ap(xt[e][u], 0, 128, 0, [[XS, KT], [1, L]]),
                    in_=_dram3(xT[e, u], 128, KT, 128 * L, L))

        def load_x_gen(e):
            load_x(e)
            yield

        def v_unit(e, j):
            """V_aug tile [jlen, 12*65]: per head 64 V cols + ones col."""
            jlen = min(128, L - j * 128)
            nc.vector.memset(
                _ap(vt[e][j], 0, 128, DH, [[VW, H], [1, 1]]), 64.0)
            for (c0, nh) in ((0, 8), (512, 4)):
                w = nh * DH
                ps = projp.tile([128, 512], F32, tag="proj",
                                name=f"vps{e}_{j}_{c0}")
                i = 0
                for (ux, uw) in ((0, 0), (0, 1), (1, 0)):
                    for kp in (0, 2, 4):
                        nc.tensor.matmul(ps[:jlen, :w],
                                         xtp(e, ux, kp, j * 128, jlen),
                                         wvp(uw, kp, c0, w),
                                         start=(i == 0), stop=(i == 8),
                                         perf_mode=DR)
                        i += 1
                        if i % 3 == 0:
                            yield
                nc.vector.tensor_tensor(
                    out=_ap(vt[e][j], 0, jlen, (c0 // DH) * VW,
                            [[VW, nh], [1, DH]]),
                    in0=_ap(ps, 0, jlen, 0, [[DH, nh], [1, DH]]),
                    in1=_ap(bv_bc, 0, jlen, c0, [[DH, nh], [1, DH]]),
                    op=ADD)
            nc.sync.dma_start(out=vo[e, j][0:jlen, :], in_=vt[e][j][:jlen, :])

        def qk_unit(e, m):
            """m 0..5: Q m-tile -> qhi (fp8, +bias). m 6..11: K m-tile ->
            khilo hi/lo planes (fp8, biasless) + straggler col + export."""
            for c in (0, 1):
                ps = projp.tile([128, 512], F32, tag="proj",
                                name=f"qkps{e}_{m}_{c}")
                i = 0
                for (uw, ux) in ((0, 0), (1, 0), (0, 1)):
                    for kp in (0, 2, 4):
                        nc.tensor.matmul(ps[:, :],
                                         wqkp(uw, kp, m * 128, 128),
                                         xtp(e, ux, kp, c * 512, 512),
                                         start=(i == 0), stop=(i == 8),
                                         perf_mode=DR)
                        i += 1
                        if i % 3 == 0:
                            yield
                if m < KT:
                    nc.vector.tensor_scalar(
                        out=_ap(qhi[e][m], 0, 128, c * 512, [[1, 512]]),
                        in0=ps[:, :], scalar1=1.0 / 64.0,
                        scalar2=_ap(bq_t, 0, 128, m, [[1, 1]]),
                        op0=MULT, op1=ADD)
                else:
                    nc.vector.tensor_scalar(
                        out=_ap(khilo[e][m - KT], 0, 128, c * 512,
                                [[1, 512]]),
                        in0=ps[:, :], scalar1=1.0 / 64.0, scalar2=None,
                        op0=MULT)
            if m >= KT:
                kh = khilo[e][m - KT]
                ps = projp.tile([128, 512], F32, tag="proj",
                                name=f"qksg{e}_{m}")
                i = 0
                for (uw, ux) in ((0, 0), (1, 0), (0, 1)):
                    for kp in (0, 2, 4):
                        nc.tensor.matmul(ps[:, 0:1],
                                         wqkp(uw, kp, m * 128, 128),
                                         xtp(e, ux, kp, 1024, 1),
                                         start=(i == 0), stop=(i == 8),
                                         perf_mode=DR)
                        i += 1
                yield
                nc.vector.tensor_scalar(
                    out=_ap(kh, 0, 128, 1024, [[1, 1]]),
                    in0=ps[:, 0:1], scalar1=1.0 / 64.0, scalar2=None,
                    op0=MULT)
                nc.sync.dma_start(out=kTo[e, m - KT], in_=kh[:])

        def o_unit(e, m, big=False, korder=tuple(range(KT))):
            yt = ytpool.tile([128, 1024], BF16, tag="yt", name=f"yt{e}_{m}")
            if big:
                psb = bigp.tile([128, 1024], F32, tag="big",
                                name=f"opsb{e}_{m}")
            for c in (0, 1):
                if big:
                    ps = psb[:, c * 512:c * 512 + 512]
                else:
                    ps = projp.tile([128, 512], F32, tag="proj",
                                    name=f"ops{e}_{m}_{c}")[:, :]
                for i, k in enumerate(korder):
                    if e == 0:
                        rhs = _ap(oTt[0], 0, 128, k * 1024 + c * 512,
                                  [[1, 512]])
                    else:
                        rhs = _ap(oTt[1][k], 0, 128, c * 512, [[1, 512]])
                    nc.tensor.matmul(
                        ps, wo(k, m * 128, 128), rhs,
                        start=(i == 0), stop=(i == KT - 1))
                    if i % 2 == 1:
                        yield
                nc.vector.tensor_scalar_add(
                    out=yt[:, c * 512:c * 512 + 512], in0=ps,
                    scalar1=_ap(bo_t, 0, 128, m, [[1, 1]]))
            nc.sync.dma_start(out=yT[e, m], in_=yt[:])

        class Fill:
            def __init__(self, gens, pri=()):
                self.gens = list(gens)
                self.pri = list(pri)

            def pull(self, n=1):
                while n > 0 and self.pri:
                    try:
                        next(self.pri[0])
                        n -= 1
                    except StopIteration:
                        self.pri.pop(0)
                while n > 0 and self.gens:
                    try:
                        next(self.gens[0])
                        n -= 1
                    except StopIteration:
                        self.gens.pop(0)

            def finish_pri(self):
                for gen in self.pri:
                    for _ in gen:
                        pass
                self.pri = []

            def finish(self, k):
                for gen in self.gens[:k]:
                    for _ in gen:
                        pass
                self.gens = self.gens[k:]

            def append(self, gen):
                self.gens.append(gen)

            def flush(self):
                self.finish(len(self.gens))

        # ---- software-pipelined attention: head h's score/exp loop
        # overlaps head h-1's PV accumulation so ACT never idles ----
        state = {"pend": None, "pts": None, "pv": None}

        def pv_step(qg):
            """One q-block-group of the pending head's PV: 8 consecutive
            matmuls accumulating over all key blocks. Groups sharing a
            psum bank must be consecutive - a start=True clears the whole
            bank's has_written bits on hardware."""
            e2, h2 = state["pend"]
            pts = state["pts"]
            pva, pvb = state["pv"]
            pv = pva if qg < 4 else pvb
            qb = qg % 4
            for j in range(NJ):
                nc.tensor.matmul(
                    pv[:, qb * VW:qb * VW + VW],
                    pts[j][:, qg * 128:qg * 128 + 128],
                    vt[e2][j][:, h2 * VW:h2 * VW + VW],
                    start=(j == 0), stop=(j == NJ - 1))

        def finalize_pend():
            """Normalize the pending head: rec = 1/(D8+p8); O_sb = pv*rec."""
            e2, h2 = state["pend"]
            pva, pvb = state["pv"]
            for c, pv in ((0, pva), (1, pvb)):
                nc.vector.tensor_tensor(
                    out=_ap(dnF[e2], 0, 128, h2 * 8 + c * 4, [[1, 4]]),
                    in0=_ap(pv, 0, 128, DH, [[VW, 4]]),
                    in1=_ap(p8[e2], 0, 128, h2 * 8 + c * 4, [[1, 4]]),
                    op=ADD)
                rec = recpool.tile([128, 4], F32, tag="rec",
                                   name=f"rec{e2}_{h2}_{c}")
                nc.vector.reciprocal(rec[:, :],
                                     _ap(dnF[e2], 0, 128, h2 * 8 + c * 4,
                                         [[1, 4]]))
                nc.vector.tensor_tensor(
                    out=_ap(osb[e2], 0, 128, c * 4 * D + h2 * 64,
                            [[D, 4], [1, 64]]),
                    in0=_ap(pv, 0, 128, 0, [[VW, 4], [1, 64]]),
                    in1=_ap(rec, 0, 128, 0, [[1, 4], [0, 64]]), op=MULT)
            if e2 == 1 and h2 % 2 == 1:
                m = h2 // 2
                for qb in range(8):
                    nc.sync.dma_start(
                        out=_ap(oTt[1][m], 0, 128, qb * 128, [[1, 128]]),
                        in_=_ap(osb[1], 0, 128, qb * D + m * 128, [[1, 128]]),
                        transpose=True)
            if e2 == 0 and h2 == H - 1:
                for qb in range(8):
                    nc.sync.dma_start(
                        out=_ap(oTt[0], 0, 128, qb * 128,
                                [[1024, KT], [1, 128]]),
                        in_=_ap(osb[0], 0, 128, qb * D, [[1, D]]),
                        transpose=True)
            state["pend"] = None

        def s8_pair(e, mp):
            """Straggler-key scores + exp for head pair (2mp, 2mp+1), as a
            filler unit through the proj psum pool."""
            ps = projp.tile([128, 512], F32, tag="proj", name=f"s8_{e}_{mp}")
            for u in (0, 1):
                h = 2 * mp + u
                poff = u * 64
                kh, qh = khilo[e][mp], qhi[e][mp]
                for qb in range(8):
                    nc.tensor.matmul(
                        _ap(ps, 0, 128, u * 8 + qb, [[1, 1]]),
                        _ap(qh, poff, 64, qb * 128, [[1, 128]]),
                        _ap(kh, poff, 64, 1024, [[1, 1]]),
                        start=True, stop=True)
                yield
            # p8 = exp(s8/8) via the Schraudolph bit trick on DVE: the
            # bf16 bit pattern of e^(x/8) is ~ int16(23.083*x + 16248.8);
            # ~3% relative error on a term whose softmax weight is ~1e-3
            nc.vector.tensor_scalar(
                out=_ap(p8[e], 0, 128, 2 * mp * 8,
                        [[1, 16]]).bitcast(mybir.dt.int16),
                in0=_ap(ps, 0, 128, 0, [[1, 16]]),
                scalar1=float(128.0 / (8.0 * np.log(2.0))),
                scalar2=17016.8, op0=MULT, op1=ADD)

        def head_loop(e, h, fill):
            mq, poff = h // 2, (h % 2) * 64
            kh, qh = khilo[e][mq], qhi[e][mq]
            # scores: one DoubleRow matmul per (j, q-half):
            # S^T = Khi^T.Qhi + Klo^T.Qhi; the previous head's PV rides
            # along one q-block group per step, two steps delayed so its
            # psum-slot WAR on the normalize never blocks the queue
            pts = []
            for j in range(NJ):
                sps = bigp.tile([128, 1024], F32, tag="big",
                                name=f"sps{e}_{h}_{j}")
                for c in (0, 1):
                    nc.tensor.matmul(
                        sps[:, c * 512:c * 512 + 512],
                        _ap(kh, poff, 64, j * 128, [[0, 2], [1, 128]]),
                        _ap(qh, poff, 64, c * 512, [[0, 2], [1, 512]]),
                        start=True, stop=True, perf_mode=DR)
                pt = ptpool.tile([128, 1024], BF16, tag="pt",
                                 name=f"pt{e}_{h}_{j}")
                nc.scalar.activation(pt[:, :], sps[:, :], EXP,
                                     bias=0.0, scale=float(SCALE * 0.5))
                pts.append(pt)
                if state["pend"] is not None and j >= 2:
                    pv_step(j - 2)
                fill.pull(2)
            if state["pend"] is not None:
                pv_step(6)
                pv_step(7)
                finalize_pend()
            pva = pvp.tile([128, 260], F32, tag="pv", name=f"pv{e}_{h}_0")
            pvb = pvp.tile([128, 260], F32, tag="pv", name=f"pv{e}_{h}_1")
            state["pend"] = (e, h)
            state["pts"] = pts
            state["pv"] = (pva, pvb)
            fill.pull(1)

        def drain_pend(fill):
            for qg in range(NJ):
                pv_step(qg)
                fill.pull(1)
            finalize_pend()

        # ---- schedule ----
        # warm the exp table during the input DMA shadow
        warm = recpool.tile([1, 4], F32, tag="warm", name="warm")
        nc.vector.memset(warm[:1, 0:1], 0.0)
        nc.scalar.activation(warm[:1, 0:1], warm[:1, 0:1], EXP,
                             bias=0.0, scale=1.0)
        for u in (0, 1):
            base = w_qk[u]
            for m in (0, KT):
                nc.sync.dma_start(
                    out=_ap(wqk_t[u], 0, 128, m * 128,
                            [[2 * D, KT], [1, 128]]),
                    in_=bass.AP(tensor=base.tensor,
                                offset=base.offset + m * 128,
                                ap=[[2 * D, 128], [128 * 2 * D, KT],
                                    [1, 128]]))
        load_x(0)
        nc.sync.dma_start(out=_ap(bq_t, 0, 128, 0, [[1, KT]]),
                          in_=_dram3(b_q[:, :], 128, KT, 128, 1))
        for u in (0, 1):
            nc.sync.dma_start(out=_ap(wv_t[u], 0, 128, 0, [[D, KT], [1, D]]),
                              in_=_dram3(w_v[u], 128, KT, 128 * D, D))
        bva = b_v[:]
        nc.sync.dma_start(out=bv_bc[:], in_=bass.AP(
            tensor=bva.tensor, offset=bva.offset,
            ap=[[0, 128], list(bva.ap[1])]))
        nc.sync.dma_start(out=_ap(bo_t, 0, 128, 0, [[1, KT]]),
                          in_=_dram3(b_o[:, :], 128, KT, 128, 1))
        nc.sync.dma_start(out=_ap(wo_t, 0, 128, 0, [[D, KT], [1, D]]),
                          in_=_dram3(w_o[:, :], 128, KT, 128 * D, D))

        def run(gen):
            for _ in gen:
                pass

        # keep the PE continuously busy through the input-DMA window so
        # the first real matmuls are costed at full pstate
        wup = consts.tile([128, 512], BF16, name="wup")
        nc.vector.memset(wup[:], 0.0)
        wps = projp.tile([128, 512], F32, tag="proj", name="wps")
        for i in range(14):
            nc.tensor.matmul(wps[:, :], wup[:, 0:128], wup[:, :],
                             start=(i == 0), stop=(i == 13))

        alloc_elem(0)
        run(qk_unit(0, 0))
        run(qk_unit(0, 6))
        run(s8_pair(0, 0))
        for u in (0, 1):
            base = w_qk[u]
            for a in (128, 7 * 128):
                nc.sync.dma_start(
                    out=_ap(wqk_t[u], 0, 128, a, [[2 * D, KT], [1, 5 * 128]]),
                    in_=bass.AP(tensor=base.tensor, offset=base.offset + a,
                                ap=[[2 * D, 128], [128 * 2 * D, KT],
                                    [1, 5 * 128]]))
        for j in range(5):
            run(v_unit(0, j))

        pri = [v_unit(0, j) for j in range(5, 9)]
        gens = []
        for m in range(1, KT):
            gens += [qk_unit(0, m), qk_unit(0, KT + m), s8_pair(0, m)]
        gens += [load_x_gen(1)]
        alloc_elem(1)
        gens += [v_unit(1, j) for j in range(9)]
        gens += [qk_unit(1, 5), qk_unit(1, KT + 5), s8_pair(1, 5)]
        gens += [qk_unit(1, 0), qk_unit(1, KT), s8_pair(1, 0)]
        fill = Fill(gens, pri=pri)
        for h in range(H):
            if h == 1:
                fill.finish_pri()
            if h >= 2 and h % 2 == 0:
                fill.finish(3)
            head_loop(0, h, fill)
        fill.flush()
        # element 1's attention: its scores pipeline over element 0's
        # last PV; element 0's output projection (needs ALL of O^T(0),
        # finalized inside head (1,0)) fills from head (1,1) on
        gens2 = []
        for m in range(1, KT - 1):
            gens2 += [qk_unit(1, m), qk_unit(1, KT + m), s8_pair(1, m)]
        fill2 = Fill(gens2)
        hseq = [10, 11] + list(range(10))
        for i, h in enumerate(hseq):
            if i >= 4 and i % 2 == 0:
                fill2.finish(3)
            head_loop(1, h, fill2)
            if i == 1:
                for m in range(KT):
                    fill2.append(o_unit(0, m))
        drain_pend(fill2)
        for e in (0, 1):
            nc.sync.dma_start(out=p8o[e], in_=p8[e][:])
            nc.sync.dma_start(out=dno[e], in_=dnF[e][:])
        fill2.flush()
        KO = (0, 1, 2, 3, 5, 4)
        for m0 in range(0, KT, 2):
            pair = [o_unit(1, m0, big=True, korder=KO),
                    o_unit(1, m0 + 1, big=True, korder=KO)]
            while pair:
                for g in pair[:]:
                    try:
                        next(g)
                    except StopIteration:
                        pair.remove(g)


def _hilo(a):
    hi = a.astype(NPF8)
    lo = (a - hi.astype(np.float32)).astype(NPF8)
    return np.stack([hi, lo])


def _prep_inputs(query, Wqkv, bqkv, Wo, bo):
    Wp = Wqkv.reshape(D, 3, DH, H).transpose(0, 1, 3, 2).reshape(D, 3 * D)
    bp = bqkv.reshape(3, DH, H).transpose(0, 2, 1).reshape(3 * D)
    w_qk = _hilo(64.0 * Wp[:, :2 * D])
    w_v = _hilo(64.0 * Wp[:, 2 * D:])
    w_o = np.ascontiguousarray(Wo).astype(ml_dtypes.bfloat16)
    b_q = np.ascontiguousarray(bp[:D]).astype(np.float32).reshape(D, 1)
    b_v = (64.0 * bp[2 * D:]).astype(np.float32).reshape(1, D)
    b_o = np.ascontiguousarray(bo).astype(np.float32).reshape(D, 1)

    in_maps = []
    for c in range(N_CORES):
        xc = query[c * BPC:(c + 1) * BPC]
        xTc = np.ascontiguousarray(xc.transpose(0, 2, 1))
        in_maps.append(dict(xT=_hilo(xTc).transpose(1, 0, 2, 3).copy(),
                            w_qk=w_qk, w_v=w_v, w_o=w_o,
                            b_q=b_q, b_v=b_v, b_o=b_o))
    return in_maps


def kernel(query, Wqkv, bqkv, Wo, bo):
    query = np.asarray(query, dtype=np.float32)
    Wqkv = np.asarray(Wqkv, dtype=np.float32)
    bqkv = np.asarray(bqkv, dtype=np.float32)
    Wo = np.asarray(Wo, dtype=np.float32)
    bo = np.asarray(bo, dtype=np.float32)

    if "nc" not in _CACHE:
        _CACHE["nc"] = _build()
    nc = _CACHE["nc"]

    in_maps = _prep_inputs(query, Wqkv, bqkv, Wo, bo)
    res = run_bass_kernel_spmd(nc, in_maps, core_ids=list(range(N_CORES)))

    Wp = Wqkv.reshape(D, 3, DH, H).transpose(0, 1, 3, 2).reshape(D, 3 * D)
    bp = bqkv.reshape(3, DH, H).transpose(0, 2, 1).reshape(3 * D)
    out = np.empty((B, L, D), dtype=np.float32)
    for c in range(N_CORES):
        r = res.results[c]
        for e in range(BPC):
            b = c * BPC + e
            # main output: Y^T tiles [6, 128, 1024] -> Y [1024, 768]
            y = np.asarray(r["yT"][e], dtype=np.float32).reshape(
                D, 1024).T.copy()
            # rank-12 straggler-key correction: Y += (p8/D) @ (v_1024 Wo_h)
            p8v = np.asarray(r["p8o"][e], dtype=np.float32)
            dnv = np.asarray(r["dno"][e], dtype=np.float32)
            p8n = (p8v / dnv).reshape(128, H, 8).transpose(2, 0, 1).reshape(
                1024, H)
            vfull = np.asarray(r["vo"][e], dtype=np.float32).reshape(
                9 * 128, H, VW)[:, :, :DH] * (1.0 / 64.0)
            v1024 = vfull[1024]  # [H, 64]
            w8v = np.einsum("hd,hde->he", v1024, Wo.reshape(H, DH, D))
            y += p8n @ w8v
            out[b, :1024] = y
            # straggler query row: exact host attention from exported K/V
            kt8 = np.asarray(r["kTo"][e], dtype=np.float32)  # [6, 128, 1040]
            kT = kt8[:, :, :L].reshape(D, L)
            qrow = query[b, L - 1] @ Wp[:, :D] + bp[:D]
            orow = np.empty(D, dtype=np.float32)
            for h in range(H):
                kh = kT[h * DH:(h + 1) * DH]  # [64, L]
                sh = (qrow[h * DH:(h + 1) * DH] @ kh) * SCALE
                ph = np.exp(sh - sh.max())
                vh = vfull[:L, h]
                orow[h * DH:(h + 1) * DH] = (ph @ vh) / ph.sum()
            out[b, L - 1] = orow @ Wo + bo
    return out
